# revision 3
# baseline (speedup 1.0000x reference)
"""Causal self-attention (B=4, N=2048, D=1024, H=16) on 8 Trainium2 NeuronCores.

Sharding: 8 cores = 4 batches x 2 head-groups (tensor-parallel over heads,
data-parallel over batch). Each core runs one SPMD Bass kernel computing, for
its (batch b, head-group g of 8 heads):

  - q/k/v projections as fp8e4m3 DoubleRow matmuls (4x bf16 column rate).
    Inputs are split host-side into hi+lo e4m3 residual pairs (x = xh + xl,
    W*32 = Wh + Wl); the three products xh@Wh + xl@Wh + xh@Wl restore
    ~bf16 accuracy at 0.75x the fp8 pair cost (the dropped xl@Wl term is
    O(eps^2)). The 2^5 weight prescale keeps the residual above the e4m3
    subnormal flush; it is undone on the psum->sbuf copy.
  - causal attention per head in S^T layout ([k partitions, q free]):
    scores^T bf16 via TensorE (contraction HD=64), exp on ScalarE with
    scale=1/8 and a constant -4 shift folded in. The diagonal 128-block of
    attexp stays bf16 (mask-multiplied) and hits a bf16 [v|32] matmul; all
    strictly-below-diagonal attexp is written as fp8e5m2 and consumed by
    DoubleRow matmuls whose lhsT packs (v_hi, v_lo) e4m3 subtiles
    [v64 | 32*ones | 0...] (M=128: dual-fp8 ldweights requires pow2 cols)
    against a stride-0-replicated ae rhs. Row 64 of the o-psum is then
    32*softmax-denominator; rows 65-127 are never read. Per-q
    normalization: reciprocal on VectorE, partition-broadcast via a
    DRAM-bounce DMA, multiply on VectorE.
  - the partial output projection out_p = sa_g @ Wproj[:, g-cols].T in bf16.

The host sums the two bf16 head-group partials per batch and adds bproj plus
the (bkqv_v @ Wproj.T) correction (the v-bias is folded out of the kernel).
"""
import os

import numpy as np
import ml_dtypes
import bass_rust

import concourse.bass as bass
import concourse.mybir as mybir
import concourse.tile as tile_mod

from concourse.tile import TileContext
from concourse.vector_clock import ScopedClock
from concourse.bass_utils import run_bass_kernel_spmd

F32 = mybir.dt.float32
BF16 = mybir.dt.bfloat16
E4 = mybir.dt.float8e4
E5 = mybir.dt.float8e5
AF = mybir.ActivationFunctionType
DR = mybir.MatmulPerfMode.DoubleRow
ALU = mybir.AluOpType

B = 4         # batch
N = 2048      # sequence length
D = 1024      # model dim
HD = 64       # head dim
HLOC = 8      # heads per core
NPAIR = 4     # head pairs per core
DSUB = 8      # D / 128 contraction subtiles
NCH = 16      # N / 128 chunks
QC = 4        # N / 512 q-chunks
EXP_SHIFT = -4.0   # exp(s/8 - 4): folded constant, cancels in softmax ratio
VSCALE = 32.0      # v psum carries 32*v; ones-row = 32 so recip = 1/(32*den)


def _patch_tile_drain():
    """The walrus build in this container rejects sync waits attached to an SP
    Drain (setupSyncWait<CTRL_NO_STRUCT>); emit one wait_ge per semaphore
    before a bare drain instead."""
    if getattr(tile_mod.TileContext, "_drain_patched", False):
        return

    def _drain_and_barrier(self, tick_clock, wait_clock):
        probe = mybir.InstNoOp(name="I-drainprobe", ins=[], outs=[])
        probe.engine = mybir.EngineType.SP
        wait_clock.add_sem_waits(probe, ScopedClock({None: tick_clock.global_clock}))
        sem_by_num = {h.num: h for h in self.sems.allocated().values()}
        for w in (probe.sync_info.on_wait if probe.sync_info else []):
            self.nc.sync.wait_ge(sem_by_num[w.id], w.wait_value)
        self.nc.sync.drain()
        self.nc.all_engine_barrier()
        popped = self.nc._tile_sem_poison_stack.pop()
        assert popped is self._sem_poison
        self.nc.clear_and_free_semaphores(list(self.sems.allocated().values()))
        self.nc.all_engine_barrier()

    tile_mod.TileContext._drain_and_barrier = _drain_and_barrier
    tile_mod.TileContext._drain_patched = True


def _split_excess_waits(nc, max_waits=1):
    """This walrus accepts at most one sync wait per instruction; hoist the
    rest onto standalone EventSemaphore waits on the same engine stream
    (waits fire at issue time, so ordering semantics are identical)."""
    idx = 0
    for fn in nc.m.functions:
        for blk in fn.blocks:
            out = []
            for inst in blk.instructions:
                si = inst.sync_info
                waits = list(si.on_wait) if (si and si.on_wait) else []
                if len(waits) > max_waits:
                    for w in waits[:-max_waits]:
                        ev = mybir.InstEventSemaphore(
                            name=f"I-wsplit{idx}", ins=[], outs=[])
                        idx += 1
                        ev.engine = inst.engine
                        ev.sync_info = bass_rust.SyncInfo(on_wait=[w],
                                                          on_update=[])
                        nc.register_instruction(ev, overwrite=True)
                        out.append(ev)
                    si.on_wait = waits[-max_waits:]
                out.append(inst)
            blk.instructions = out


def _stride0_pair(ap):
    """Replicate an AP along a new dim1 of size 2 with stride 0 (the same
    data feeds both DoubleRow subtiles)."""
    return bass.AP(tensor=ap.tensor, offset=ap.offset,
                   ap=[ap.ap[0], [0, 2]] + list(ap.ap[1:]))


def build_kernel():
    _patch_tile_drain()
    nc = bass.Bass("TRN2")

    xh = nc.dram_tensor("xh", [DSUB, 128, N], E4, kind="ExternalInput")
    xl = nc.dram_tensor("xl", [DSUB, 128, N], E4, kind="ExternalInput")
    wqkh = nc.dram_tensor("wqkh", [128, NPAIR, 2, DSUB, 128], E4, kind="ExternalInput")
    wqkl = nc.dram_tensor("wqkl", [128, NPAIR, 2, DSUB, 128], E4, kind="ExternalInput")
    wvh = nc.dram_tensor("wvh", [128, DSUB, HLOC * HD], E4, kind="ExternalInput")
    wvl = nc.dram_tensor("wvl", [128, DSUB, HLOC * HD], E4, kind="ExternalInput")
    wproj = nc.dram_tensor("wproj", [128, 4, D], BF16, kind="ExternalInput")
    bqk = nc.dram_tensor("bqk", [128, NPAIR * 2], F32, kind="ExternalInput")
    out_p = nc.dram_tensor("out_p", [N, D], BF16, kind="ExternalOutput")

    # causal keep-mask for the diagonal 128x128 block of S^T: keep q >= k
    mask_np = np.triu(np.ones((128, 128), np.float32)).astype(ml_dtypes.bfloat16)
    maskt = nc.inline_tensor(mask_np, name="diagmask")

    with TileContext(nc) as tc:
        with (
            tc.tile_pool(name="persist", bufs=1) as persist,
            tc.tile_pool(name="qk", bufs=2) as qkpool,
            tc.tile_pool(name="ae", bufs=8) as aepool,
            tc.tile_pool(name="aed", bufs=8) as aedpool,
            tc.tile_pool(name="norm", bufs=8) as normpool,
            tc.tile_pool(name="outst", bufs=4) as outpool,
            tc.tile_pool(name="dscr", bufs=8, space="DRAM") as dscr,
            tc.tile_pool(name="acc512", bufs=2, space="PSUM") as acc512,
            tc.tile_pool(name="scps", bufs=2, space="PSUM") as scps,
            tc.tile_pool(name="oax", bufs=2, space="PSUM") as oax,
        ):
            # ---- persistent SBUF tensors ----
            xhb = persist.tile([128, DSUB, N], E4, tag="xhb")
            xlb = persist.tile([128, DSUB, N], E4, tag="xlb")
            # v8[:, c, term, 128h:128h+128] = [v64 | 32-ones | 63 zeros] e4m3
            v8 = persist.tile([128, NCH, 2, HLOC * 128], E4, tag="v8")
            # vsb bf16 [v64 | 32] per head, for the diagonal-block matmuls
            vsb = persist.tile([128, NCH, HLOC * (HD + 1)], BF16, tag="vsb")
            sasb = persist.tile([128, NPAIR, N], BF16, tag="sasb")
            wvhsb = persist.tile([128, DSUB, HLOC * HD], E4, tag="wvhsb")
            wvlsb = persist.tile([128, DSUB, HLOC * HD], E4, tag="wvlsb")
            wprojsb = persist.tile([128, 4, D], BF16, tag="wprojsb")
            wqkhsb = persist.tile([128, NPAIR, 2, DSUB, 128], E4, tag="wqkhsb")
            wqklsb = persist.tile([128, NPAIR, 2, DSUB, 128], E4, tag="wqklsb")
            bqksb = persist.tile([128, NPAIR * 2], F32, tag="bqksb")
            masksb = persist.tile([128, 128], BF16, tag="masksb")
            shiftc = persist.tile([128, 1], F32, tag="shiftc")

            # ---- phase A0: PE warm-up ----
            # Dummy matmuls on a memset tile run during the initial DMA wait,
            # releasing the HAM clock gate so the first real matmuls issue at
            # full rate.
            warm = persist.tile([128, 128], BF16, tag="warm")
            nc.vector.memset(warm[:], 0.0)
            wps = scps.tile([128, 1024], F32, tag="sc", name="warmps")
            for i in range(48):
                nc.tensor.matmul(wps[:, 0:128], lhsT=warm[:], rhs=warm[:],
                                 start=True, stop=True)

            # ---- phase A: loads ----
            # x hi/lo stream on the SP and Pool queues split per (s,
            # n-quarter) so the v/qk matmuls start as soon as the first
            # quarter lands; weights go down the ScalarE DMA queue.
            nc.vector.memset(shiftc[:], EXP_SHIFT)
            nc.scalar.dma_start(wvhsb[:, 0:1, :], wvh[:, 0:1, :])
            nc.scalar.dma_start(wvlsb[:, 0:1, :], wvl[:, 0:1, :])
            for nq in range(4):
                for s in range(DSUB):
                    nc.sync.dma_start(xhb[:, s, nq * 512:(nq + 1) * 512],
                                      xh[s, :, nq * 512:(nq + 1) * 512])
                    nc.gpsimd.dma_start(xlb[:, s, nq * 512:(nq + 1) * 512],
                                        xl[s, :, nq * 512:(nq + 1) * 512])
                if nq == 0:
                    nc.scalar.dma_start(wvhsb[:, 1:DSUB, :], wvh[:, 1:DSUB, :])
                    nc.scalar.dma_start(wvlsb[:, 1:DSUB, :], wvl[:, 1:DSUB, :])
            for p in range(NPAIR):  # per-pair so C1(p=0) unblocks early
                nc.scalar.dma_start(wqkhsb[:, p], wqkh[:, p])
                nc.scalar.dma_start(wqklsb[:, p], wqkl[:, p])
            nc.sync.dma_start(masksb[:], maskt[:])
            nc.sync.dma_start(bqksb[:], bqk[:])
            nc.sync.dma_start(wprojsb[:], wproj[:])

            # ---- phase B: v for all 8 heads ----
            v8v = v8[:].rearrange("p c t (h e) -> p c t h e", e=128)
            vview = vsb[:].rearrange("p c (h e) -> p c h e", e=HD + 1)
            for nch in range(NCH):
                # init the ones/garbage cols of this chunk (cols 64-127 per
                # head-block): zeros, then 32.0 in the term-0 ones col
                nc.gpsimd.memset(v8v[:, nch, :, :, HD:128], 0.0)
                nc.gpsimd.memset(v8v[:, nch, 0:1, :, HD:HD + 1], VSCALE)
                nc.vector.memset(vview[:, nch, :, HD:HD + 1], VSCALE)
                ps = acc512.tile([128, 512], F32, tag="acc")
                k = 0
                for xa, wv_ in ((xhb, wvhsb), (xlb, wvhsb), (xhb, wvlsb)):
                    for s in range(0, DSUB, 2):
                        nc.tensor.matmul(
                            ps[:],
                            lhsT=xa[:, s:s + 2, nch * 128:(nch + 1) * 128],
                            rhs=wv_[:, s:s + 2, :],
                            start=(k == 0), stop=(k == 11), perf_mode=DR,
                        )
                        k += 1
                psv = ps[:].rearrange("p (h e) -> p h e", e=HD)
                # v8 hi = e4m3(32v); v8 lo = e4m3(32v - hi); vsb = bf16(32v)
                nc.vector.tensor_copy(v8v[:, nch, 0, :, 0:HD], psv)
                nc.vector.tensor_tensor(v8v[:, nch, 1, :, 0:HD], psv,
                                        v8v[:, nch, 0, :, 0:HD],
                                        op=ALU.subtract)
                nc.vector.tensor_copy(vview[:, nch, :, 0:HD], psv)

            # ---- phase C: per head-pair q/k projection + attention ----
            for p in range(NPAIR):
                # C1: q/k stacks (rows 0-63 head A, rows 64-127 head B)
                stacks = [qkpool.tile([128, N], BF16, tag=f"qk{w}",
                                      name=f"qk{w}_{p}") for w in range(2)]
                for qc in range(QC):       # qc-outer, w-inner: the first
                    for w in range(2):     # score tile's inputs finish first
                        ps = acc512.tile([128, 512], F32, tag="acc")
                        k = 0
                        for wt, xa in ((wqkhsb, xhb), (wqkhsb, xlb),
                                       (wqklsb, xhb)):
                            for s in range(0, DSUB, 2):
                                nc.tensor.matmul(
                                    ps[:],
                                    lhsT=wt[:, p, w, s:s + 2, :],
                                    rhs=xa[:, s:s + 2, qc * 512:(qc + 1) * 512],
                                    start=(k == 0), stop=(k == 11),
                                    perf_mode=DR,
                                )
                                k += 1
                        # stacks = psum/32 + bias (true q/k scale restored)
                        nc.vector.tensor_scalar(
                            stacks[w][:, qc * 512:(qc + 1) * 512], ps[:],
                            1.0 / VSCALE,
                            bqksb[:, p * 2 + w:p * 2 + w + 1],
                            op0=ALU.mult, op1=ALU.add,
                        )
                qst, kst = stacks

                # C2: attention per head of the pair
                for e in range(2):
                    h = 2 * p + e
                    q_ap = qst[64 * e:64 * e + 64]
                    k_ap = kst[64 * e:64 * e + 64]
                    for half in range(2):  # q in [1024*half, 1024*(half+1))
                        o_ps = [oax.tile([128, 512], F32, tag="o",
                                         name=f"o_{h}_{half}_{qq}")
                                for qq in range(2)]
                        touched = [False, False]

                        def emit_norm(qq):
                            # normalize + write sa^T; o leaves PSUM right
                            # away (frees the bank); the slow recip -> DRAM-
                            # bounce broadcast chain then runs on SBUF tiles
                            # off the critical path.
                            qca = 2 * half + qq
                            osb = normpool.tile([HD + 1, 512], F32, tag="osb",
                                                name=f"osb_{h}_{qca}")
                            nc.vector.tensor_copy(osb[:], o_ps[qq][0:HD + 1, :])
                            recip = normpool.tile([1, 512], F32, tag="recip",
                                                  name=f"rc_{h}_{qca}")
                            nc.vector.reciprocal(recip[:], osb[HD:HD + 1, :])
                            rdr = dscr.tile([1, 512], F32, tag="rdr",
                                            name=f"rd_{h}_{qca}")
                            nc.gpsimd.dma_start(rdr[:], recip[:])
                            rbc = normpool.tile([64, 512], F32, tag="rbc",
                                                name=f"rb_{h}_{qca}")
                            bcast_src = bass.AP(
                                tensor=rdr.tensor, offset=rdr.offset,
                                ap=[[0, 64]] + list(rdr.ap[1:]),
                            )
                            nc.gpsimd.dma_start(rbc[:], bcast_src)
                            nc.vector.tensor_mul(
                                sasb[64 * e:64 * e + 64, p,
                                     512 * qca:512 * qca + 512],
                                osb[0:HD, :], rbc[:],
                            )

                        for t in range(8 * half + 8):
                            pstart = max(128 * t, 1024 * half)
                            wp = 1024 * half + 1024 - pstart
                            sc = scps.tile([128, 1024], F32, tag="sc")
                            offs = []
                            off = 0
                            while off < wp:
                                offs.append(off); off += 512
                            for off in reversed(offs):
                                mv = min(512, wp - off)
                                nc.tensor.matmul(
                                    sc[:, off:off + mv],
                                    lhsT=k_ap[:, 128 * t:128 * t + 128],
                                    rhs=q_ap[:, pstart + off:pstart + off + mv],
                                    start=True, stop=True,
                                )
                            is_diag = pstart == 128 * t
                            estart = 128 * t + 128 if is_diag else pstart
                            # the first chunk of half 0 must cover stripe 0
                            # in ONE start=True write (real PSUM only zeroes
                            # written addresses): zero-pad ae5 down to col 0
                            pad0 = half == 0 and t == 0
                            aeo = 0 if pad0 else estart  # ae5 col-0 q-origin
                            we = 1024 * half + 1024 - estart
                            if we > 0:
                                ae5 = aepool.tile([128, 1024], E5, tag="ae")
                                if pad0:
                                    nc.gpsimd.memset(ae5[:, 0:128], 0.0)
                                nc.scalar.activation(
                                    ae5[:, estart - aeo:estart - aeo + we],
                                    sc[:, estart - pstart:estart - pstart + we],
                                    AF.Exp, scale=0.125, bias=shiftc[:])
                                for qq in (1, 0):
                                    qca = 2 * half + qq
                                    q_lo = max(aeo, 512 * qca)
                                    q_hi = 512 * qca + 512
                                    if q_lo >= q_hi:
                                        continue
                                    wN = q_hi - q_lo
                                    nc.tensor.matmul(
                                        o_ps[qq][:, q_lo - 512 * qca:512],
                                        lhsT=v8[:, t, :, 128 * h:128 * h + 128],
                                        rhs=_stride0_pair(
                                            ae5[:, q_lo - aeo:q_lo - aeo + wN]),
                                        start=not touched[qq], stop=False,
                                        perf_mode=DR, skip_group_check=True,
                                    )
                                    touched[qq] = True
                            if is_diag:
                                # diagonal block: bf16 attexp, masked, bf16 av
                                aed = aedpool.tile([128, 128], BF16, tag="aed")
                                nc.scalar.activation(aed[:], sc[:, 0:128],
                                                     AF.Exp, scale=0.125,
                                                     bias=shiftc[:])
                                nc.vector.tensor_mul(aed[:], aed[:], masksb[:])
                                qca = (128 * t - 1024 * half) // 512 + 2 * half
                                qq = qca - 2 * half
                                doff = 128 * t - 512 * qca
                                nc.tensor.matmul(
                                    o_ps[qq][0:HD + 1, doff:doff + 128],
                                    lhsT=vview[:, t, :, :].rearrange(
                                        "p h e -> p (h e)")[
                                        :, h * (HD + 1):(h + 1) * (HD + 1)],
                                    rhs=aed[:],
                                    start=not touched[qq],
                                    stop=(t == 4 * qca + 3),
                                    skip_group_check=True,
                                )
                                touched[qq] = True
                                if t == 4 * qca + 3:
                                    emit_norm(qq)

            # ---- phase D: output projection (partial sum for this group) ----
            for nch in range(NCH):
                for dc in range(2):
                    ps = acc512.tile([128, 512], F32, tag="acc")
                    for j in range(4):
                        nc.tensor.matmul(
                            ps[:],
                            lhsT=sasb[:, j, nch * 128:(nch + 1) * 128],
                            rhs=wprojsb[:, j, dc * 512:(dc + 1) * 512],
                            start=(j == 0), stop=(j == 3),
                        )
                    ob = outpool.tile([128, 512], BF16, tag="ob")
                    nc.vector.tensor_copy(ob[:], ps[:])
                    oeng = nc.sync if (nch + dc) % 2 == 0 else nc.gpsimd
                    oeng.dma_start(
                        out_p[nch * 128:(nch + 1) * 128, dc * 512:(dc + 1) * 512],
                        ob[:],
                    )

    _split_excess_waits(nc)
    return nc


# ---------------- host-side sharding ----------------

def _split_e4(a):
    e4 = ml_dtypes.float8_e4m3
    hi = np.asarray(a, e4)
    lo = np.asarray(a - hi.astype(np.float32), e4)
    return hi, lo


def prep_core_inputs(x, Wkqv, bkqv, Wproj, b, g):
    """Per-core input dict for core (batch b, head-group g)."""
    bf = ml_dtypes.bfloat16
    Wg = np.asarray(Wkqv[g * HLOC:(g + 1) * HLOC], np.float32)  # [8, 192, 1024]
    bg = np.asarray(bkqv[g * HLOC:(g + 1) * HLOC], np.float32)  # [8, 192]
    Wk, Wq, Wv = Wg[:, :HD], Wg[:, HD:2 * HD], Wg[:, 2 * HD:]
    bk, bq = bg[:, :HD], bg[:, HD:2 * HD]

    xT = np.ascontiguousarray(np.asarray(x[b], np.float32).T).reshape(DSUB, 128, N)
    xh, xl = _split_e4(xT)

    # wqk[d, p, w, s, (e j)] = W(w)[2p+e, j, 128s+d] * 32
    def stack_pairs(W):  # W [8, 64, 1024] -> [128, 4, 8, 128]
        t = W.reshape(NPAIR, 2, HD, DSUB, 128)               # p e j s d
        return t.transpose(4, 0, 3, 1, 2).reshape(128, NPAIR, DSUB, 128)

    wqk32 = np.ascontiguousarray(
        np.stack([stack_pairs(Wq), stack_pairs(Wk)], axis=2)  # [128, 4, 2, 8, 128]
    ) * VSCALE
    wqkh, wqkl = _split_e4(wqk32)
    wv32 = np.ascontiguousarray(
        Wv.reshape(HLOC, HD, DSUB, 128).transpose(3, 2, 0, 1).reshape(
            128, DSUB, HLOC * HD)) * VSCALE
    wvh, wvl = _split_e4(wv32)
    wproj = np.ascontiguousarray(
        np.asarray(Wproj, np.float32).T[g * 512:(g + 1) * 512]
        .reshape(4, 128, D).transpose(1, 0, 2)).astype(bf)

    bqk = np.zeros((128, NPAIR * 2), np.float32)
    for p in range(NPAIR):
        for e in range(2):
            h = 2 * p + e
            bqk[64 * e:64 * e + 64, 2 * p + 0] = bq[h]
            bqk[64 * e:64 * e + 64, 2 * p + 1] = bk[h]

    return {"xh": xh, "xl": xl, "wqkh": wqkh, "wqkl": wqkl,
            "wvh": wvh, "wvl": wvl, "wproj": wproj, "bqk": bqk}


_NC_CACHE = {}


def _get_nc():
    if "nc" not in _NC_CACHE:
        _NC_CACHE["nc"] = build_kernel()
    return _NC_CACHE["nc"]


def kernel(x, Wkqv, bkqv, Wproj, bproj):
    x = np.asarray(x, np.float32)
    Wkqv = np.asarray(Wkqv, np.float32)
    bkqv = np.asarray(bkqv, np.float32)
    Wproj = np.asarray(Wproj, np.float32)
    bproj = np.asarray(bproj, np.float32)

    try:  # tracing needs the axon NTFF hook, absent in this container
        from antenv.axon_hooks import get_axon_ntff_profile_hook  # noqa: F401
    except ImportError:
        os.environ.setdefault("BASS_NEVER_TRACE", "1")

    in_maps = [prep_core_inputs(x, Wkqv, bkqv, Wproj, b, g)
               for b in range(B) for g in range(2)]
    nc = _get_nc()
    res = run_bass_kernel_spmd(nc, in_maps, core_ids=list(range(8)))
    parts = [res.results[i]["out_p"] for i in range(8)]

    # v-bias correction folded out of the kernel: sa lacks +bv, so add
    # concat_h(bv_h) @ Wproj.T once on the host.
    bv_full = bkqv[:, 2 * HD:].reshape(D)              # [H*HD] = [D]
    corr = bv_full @ Wproj.T + bproj                   # [D]

    out = np.empty((B, N, D), np.float32)
    for b in range(B):
        out[b] = (parts[2 * b].astype(np.float32)
                  + parts[2 * b + 1].astype(np.float32) + corr[None, :])
    return out
